# revision 5
# baseline (speedup 1.0000x reference)
"""DynamicSparseMoE Trainium2 kernel.

Math (per token t):
  logits[e'] = x[t] . gate_w[e'] + gate_b[e']        (C=2048 contraction)
  gw[e']     = 1.0 if logits[e'] > 0 else 0.0
  expert e input: xe[d] = x[t, 16*d + e]  (d=0..127; expert idx fastest in channel)
  h  = gelu(fc_w[e] @ xe + fc_b[e])                   (H=512)
  oe = proj_w[e] @ h + proj_b[e]                      (DE=128)
  out[t, 128*e + d] = gw[e] * oe[d]                   (expert-major output channels)

Strategy: data-parallel over the 16384 tokens across 8 NeuronCores (2048
tokens/core).  The host pre-transposes x into expert-major channel-transposed
layout and splits it into bf16 hi/lo halves (x = xhi + xlo exactly), so every
per-expert [de=128, tok] tile DMAs directly into SBUF as a ready bf16 matmul
operand -- zero on-device transposes or casts.  Per 512-token group:
  - gate: split-bf16 matmuls (xhi*[whi|wlo] + xlo*[whi|wlo], accumulated into
    a [32, 512] PSUM region) give ~2^-17-accurate logits, needed because the
    0/1 threshold amplifies rounding into full expert-block errors.  A
    transpose-matmul against a stacked identity [I16; I16] folds the hi+lo
    column halves while transposing; DVE is_gt yields per-token gate scalars.
  - fc: bf16 weights-stationary (rhs = xhi tile, 512 cols, LDW pipelined);
    gelu on ACT in [128, 1024]-wide instrs writing bf16 h.
  - proj: lhsT-swap -- h (already [h, tok]) is the stationary operand, bf16
    proj weights are the moving rhs, so PSUM output lands directly as
    [tok, d]: no exit transposes.
  - gated evacuation: one DVE tensor_tensor per expert with a 0-stride
    broadcast of the gate column, writing bf16 expert-major output rows.
  - DMA: one batched load per (array, group); stores go out in 4-expert
    column blocks so the final store barely trails the last evacuation;
    weights are woven between the first group's loads so PE starts early.

The whole kernel is one flat software pipeline over 64 (group, expert)
slots: fc runs two slots ahead of proj, and the NEXT group's 32 gate
matmuls are woven into the second half of each group's expert loop, so
neither PE nor ACT ever sees a group boundary.  Measured ~185 us on HW
(baseline 359 us), PE-bound at ~157 us busy; rel err ~3.7e-3.
"""

import sys

for _p in ("/opt/trn_rl_repo", "/root/.axon_site"):
    if _p not in sys.path:
        sys.path.insert(0, _p)

import ml_dtypes
import numpy as np

import concourse.mybir as mybir
from concourse import bacc
from concourse.bass_utils import run_bass_kernel_spmd
from concourse.tile import TileContext

B, T, C, E = 8, 2048, 2048, 16
DE = C // E  # 128
H = 4 * DE  # 512
NCORES = 8
NTOK = B * T  # 16384
TPC = NTOK // NCORES  # tokens per core: 2048
GROUP = 512  # tokens per group
NTAU = GROUP // 128  # 4 token-tiles per group
NGRP = TPC // GROUP  # 4 groups per core

F32 = mybir.dt.float32
BF16 = mybir.dt.bfloat16
AF = mybir.ActivationFunctionType
ALU = mybir.AluOpType
GELU = AF.Gelu

_CACHE = {}


def _build(has_fcb: bool, has_pjb: bool):
    nc = bacc.Bacc(trn_type="TRN2", num_devices=NCORES)

    xhi_d = nc.dram_tensor("xhi", [C, TPC], BF16, kind="ExternalInput").ap()
    xlo_d = nc.dram_tensor("xlo", [C, TPC], BF16, kind="ExternalInput").ap()
    gw2_d = nc.dram_tensor("gw2", [128, E * 32], BF16, kind="ExternalInput").ap()
    fcw_d = nc.dram_tensor("fcw", [128, E * H], BF16, kind="ExternalInput").ap()
    pjw_d = nc.dram_tensor("pjw", [128, E * 4 * DE], BF16, kind="ExternalInput").ap()
    ngb_d = nc.dram_tensor("ngb", [128, NTAU * E], F32, kind="ExternalInput").ap()
    idn_d = nc.dram_tensor("idn", [32, 16], F32, kind="ExternalInput").ap()
    fcb_d = nc.dram_tensor("fcb", [128, E * 4], F32, kind="ExternalInput").ap()
    pjb_d = nc.dram_tensor("pjb", [128, E * DE], BF16, kind="ExternalInput").ap()
    out_d = nc.dram_tensor("out", [TPC, C], BF16, kind="ExternalOutput").ap()

    with TileContext(nc) as tc:
        with (
            tc.tile_pool(name="wts", bufs=1) as wts,
            tc.tile_pool(name="work", bufs=2) as work,
            tc.tile_pool(name="psum", bufs=1, space="PSUM") as psum,
        ):
            # ---- small resident constants first ----
            gw2_sb = wts.tile([128, E * 32], BF16)  # [p, c*32 + (whi|wlo)]
            nc.sync.dma_start(out=gw2_sb, in_=gw2_d)
            ngb_sb = wts.tile([128, NTAU * E], F32)
            nc.sync.dma_start(out=ngb_sb, in_=ngb_d)
            idn_sb = wts.tile([32, 16], F32)
            nc.sync.dma_start(out=idn_sb, in_=idn_d)
            if has_fcb:
                fcb_sb = wts.tile([128, E * 4], F32)
                nc.sync.dma_start(out=fcb_sb, in_=fcb_d)
            if has_pjb:
                pjb_sb = wts.tile([128, E * DE], BF16)
                nc.sync.dma_start(out=pjb_sb, in_=pjb_d)
            fcw_sb = wts.tile([128, E * H], BF16)  # [de, e*512+h]
            pjw_sb = wts.tile([128, E * 4 * DE], BF16)  # [h_in_chunk, (e*4+hc)*128+d]

            # ---------- software-pipelined main loop ----------
            # Per group g: expert phase (fc -> gelu -> proj -> gated evac) is
            # ACT/PE-paced; the NEXT group's 32 gate matmuls are woven into
            # the second half of the expert loop so the PE never runs a
            # dedicated gate phase with ACT idle.
            st = [dict() for _ in range(NGRP)]

            def emit_load(g):
                t0 = g * GROUP
                xhi = work.tile([128, E * GROUP], BF16, tag="xhi", bufs=2,
                                name=f"xhi_{g}")
                xlo = work.tile([128, E * GROUP], BF16, tag="xlo", bufs=2,
                                name=f"xlo_{g}")
                st[g]["xhi"], st[g]["xlo"] = xhi, xlo
                if g == 0:
                    # xhi first (gate-hi matmuls pace with its quarters),
                    # then fcw/pjw woven into the xlo stream
                    def ld_x(arr, arr_d, q):
                        cs = slice(q * 4, (q + 1) * 4)
                        nc.sync.dma_start(
                            out=arr.rearrange("p (e t) -> p e t", e=E)[:, cs, :],
                            in_=arr_d.rearrange("(e p) t -> p e t", p=128)[
                                :, cs, t0 : t0 + GROUP
                            ],
                        )

                    def ld_w(sb, dr, q, w):
                        nc.sync.dma_start(
                            out=sb[:, q * 4 * w : (q + 1) * 4 * w],
                            in_=dr[:, q * 4 * w : (q + 1) * 4 * w],
                        )

                    for q in range(4):
                        ld_x(xhi, xhi_d, q)
                    ld_w(fcw_sb, fcw_d, 0, H)
                    ld_w(pjw_sb, pjw_d, 0, 4 * DE)
                    ld_x(xlo, xlo_d, 0)
                    ld_x(xlo, xlo_d, 1)
                    ld_w(fcw_sb, fcw_d, 1, H)
                    ld_w(pjw_sb, pjw_d, 1, 4 * DE)
                    ld_x(xlo, xlo_d, 2)
                    ld_w(fcw_sb, fcw_d, 2, H)
                    ld_w(pjw_sb, pjw_d, 2, 4 * DE)
                    ld_x(xlo, xlo_d, 3)
                    ld_w(fcw_sb, fcw_d, 3, H)
                    ld_w(pjw_sb, pjw_d, 3, 4 * DE)
                else:
                    nc.sync.dma_start(
                        out=xhi.rearrange("p (e t) -> p e t", e=E),
                        in_=xhi_d.rearrange("(e p) t -> p e t", p=128)[
                            :, :, t0 : t0 + GROUP
                        ],
                    )
                    nc.sync.dma_start(
                        out=xlo.rearrange("p (e t) -> p e t", e=E),
                        in_=xlo_d.rearrange("(e p) t -> p e t", p=128)[
                            :, :, t0 : t0 + GROUP
                        ],
                    )

            def emit_gate_mms(g, c0, c1, which="both"):
                if "ps_g" not in st[g]:
                    st[g]["ps_g"] = psum.tile(
                        [32, GROUP], F32, tag="gate", bufs=1, name=f"psg_{g}"
                    )
                ps_g = st[g]["ps_g"]
                xhi, xlo = st[g]["xhi"], st[g]["xlo"]
                for c in range(c0, c1):
                    if which in ("both", "hi"):
                        nc.tensor.matmul(
                            ps_g,
                            lhsT=gw2_sb[:, c * 32 : (c + 1) * 32],
                            rhs=xhi[:, c * GROUP : (c + 1) * GROUP],
                            start=(c == 0),
                            stop=False,
                        )
                    if which in ("both", "lo"):
                        nc.tensor.matmul(
                            ps_g,
                            lhsT=gw2_sb[:, c * 32 : (c + 1) * 32],
                            rhs=xlo[:, c * GROUP : (c + 1) * GROUP],
                            start=False,
                            stop=(c == E - 1),
                        )

            def emit_gate_thresh(g):
                gsb = work.tile([32, GROUP], F32, tag="gsb", bufs=2,
                                name=f"gsb_{g}")
                nc.vector.tensor_copy(gsb, st[g]["ps_g"])
                # transpose-matmuls against the stacked identity [I16; I16]
                # fold the hi+lo halves while transposing [32, 128] -> [128, 16]
                ps_gt = psum.tile([128, NTAU * E], F32, tag="gate", bufs=1,
                                  name=f"psgt_{g}")
                for ti in range(NTAU):
                    nc.tensor.matmul(
                        ps_gt[:, ti * E : (ti + 1) * E],
                        lhsT=gsb[:, ti * 128 : (ti + 1) * 128],
                        rhs=idn_sb,
                        start=True,
                        stop=True,
                    )
                gw = work.tile([128, NTAU * E], F32, tag="gw", bufs=2,
                               name=f"gw_{g}")
                nc.vector.tensor_tensor(gw, ps_gt, ngb_sb, ALU.is_gt)
                st[g]["gw"] = gw

            def emit_fc(g, e):
                xhi = st[g]["xhi"]
                h_sb = work.tile(
                    [128, 4 * GROUP], BF16, tag="h", bufs=4, name=f"h_{g}_{e}"
                )
                for half in range(2):
                    ps_fc = psum.tile(
                        [128, 2 * GROUP], F32, tag="fc", bufs=2,
                        name=f"psfc_{g}_{e}_{half}",
                    )
                    for hq in range(2):
                        hti = half * 2 + hq
                        nc.tensor.matmul(
                            ps_fc[:, hq * GROUP : (hq + 1) * GROUP],
                            lhsT=fcw_sb[
                                :, e * H + hti * 128 : e * H + (hti + 1) * 128
                            ],
                            rhs=xhi[:, e * GROUP : (e + 1) * GROUP],
                            start=True,
                            stop=True,
                        )
                    if has_fcb:
                        for hq in range(2):
                            hti = half * 2 + hq
                            nc.scalar.activation(
                                h_sb[:, hti * GROUP : (hti + 1) * GROUP],
                                ps_fc[:, hq * GROUP : (hq + 1) * GROUP],
                                GELU,
                                bias=fcb_sb[:, e * 4 + hti : e * 4 + hti + 1],
                                scale=1.0,
                            )
                    else:
                        nc.scalar.activation(
                            h_sb[:, half * 2 * GROUP : (half + 1) * 2 * GROUP],
                            ps_fc,
                            GELU,
                        )
                st[g].setdefault("h", {})[e] = h_sb

            def emit_proj(g, e):
                ps = psum.tile(
                    [128, GROUP], F32, tag="pj", bufs=3, name=f"pspj_{g}_{e}"
                )
                h_sb = st[g]["h"][e]
                for ti in range(NTAU):
                    for hc in range(4):
                        nc.tensor.matmul(
                            ps[:, ti * 128 : (ti + 1) * 128],
                            lhsT=h_sb[
                                :,
                                hc * GROUP + ti * 128 : hc * GROUP + (ti + 1) * 128,
                            ],
                            rhs=pjw_sb[
                                :, (e * 4 + hc) * DE : (e * 4 + hc + 1) * DE
                            ],
                            start=(hc == 0),
                            stop=(hc == 3),
                        )
                st[g]["ps_pj"] = ps

            def emit_evac(g, e):
                ps = st[g]["ps_pj"]
                gw = st[g]["gw"]
                out_sb = st[g]["out"]
                ps3 = ps.rearrange("p (ti d) -> p ti d", ti=NTAU)
                out3 = out_sb.rearrange("p (ti c) -> p ti c", ti=NTAU)[
                    :, :, e * DE : (e + 1) * DE
                ]
                gw3 = (
                    gw[:, e : e + E * (NTAU - 1) + 1 : E]
                    .unsqueeze(2)
                    .to_broadcast([128, NTAU, DE])
                )
                if has_pjb:
                    tmp = work.tile(
                        [128, GROUP], BF16, tag="tmp", bufs=2,
                        name=f"tmp_{g}_{e}",
                    )
                    nc.vector.tensor_tensor(
                        tmp.rearrange("p (ti d) -> p ti d", ti=NTAU),
                        ps3,
                        pjb_sb[:, e * DE : (e + 1) * DE]
                        .unsqueeze(1)
                        .to_broadcast([128, NTAU, DE]),
                        ALU.add,
                    )
                    nc.vector.tensor_tensor(
                        out3,
                        tmp.rearrange("p (ti d) -> p ti d", ti=NTAU),
                        gw3,
                        ALU.mult,
                    )
                else:
                    nc.vector.tensor_tensor(out3, ps3, gw3, ALU.mult)

            # prologue: group 0 load + gate (cannot hide under prior work)
            emit_load(0)
            emit_gate_mms(0, 0, E, which="hi")
            emit_fc(0, 0)
            emit_gate_mms(0, 0, 8, which="lo")
            emit_fc(0, 1)
            emit_gate_mms(0, 8, E, which="lo")
            emit_gate_thresh(0)

            # flat 64-slot pipeline across all groups: fc runs 2 slots ahead
            # globally so ACT never idles at group boundaries
            slots = [(g, e) for g in range(NGRP) for e in range(E)]
            for i, (g, e) in enumerate(slots):
                if e == 0:
                    st[g]["out"] = work.tile(
                        [128, NTAU * C], BF16, tag="out", bufs=2, name=f"osb_{g}"
                    )
                    if g + 1 < NGRP:
                        emit_load(g + 1)
                if i + 2 < len(slots):
                    g2, e2 = slots[i + 2]
                    emit_fc(g2, e2)
                emit_proj(g, e)
                emit_evac(g, e)
                # weave next group's gate matmuls into the second half of
                # the expert loop (its x tiles have landed by then)
                if g + 1 < NGRP and e >= 8:
                    emit_gate_mms(g + 1, 2 * (e - 8), 2 * (e - 8) + 2)
                if e % 4 == 3:
                    e0 = e - 3
                    nc.sync.dma_start(
                        out=out_d.rearrange("(ti p) c -> p ti c", p=128)[
                            :, NTAU * g : NTAU * (g + 1),
                            e0 * DE : (e0 + 4) * DE,
                        ],
                        in_=st[g]["out"].rearrange("p (ti c) -> p ti c", ti=NTAU)[
                            :, :, e0 * DE : (e0 + 4) * DE
                        ],
                    )
                if e == E - 1 and g + 1 < NGRP:
                    emit_gate_thresh(g + 1)

    nc.compile()
    return nc


def _prep_inputs(x, gate_w, gate_b, fc_w, fc_b, proj_w, proj_b):
    x = np.ascontiguousarray(np.asarray(x, dtype=np.float32)).reshape(NTOK, C)
    gate_w = np.asarray(gate_w, dtype=np.float32)
    gate_b = np.asarray(gate_b, dtype=np.float32)
    fc_w = np.asarray(fc_w, dtype=np.float32)
    fc_b = np.asarray(fc_b, dtype=np.float32)
    proj_w = np.asarray(proj_w, dtype=np.float32)
    proj_b = np.asarray(proj_b, dtype=np.float32)

    # permuted channel order: c' = e*128 + d  ->  orig c = 16*d + e
    cp = np.arange(C)
    orig = 16 * (cp % DE) + cp // DE
    gwp = np.ascontiguousarray(gate_w[:, orig].T)  # [C', E]
    whi = gwp.astype(ml_dtypes.bfloat16)
    wlo = (gwp - whi.astype(np.float32)).astype(ml_dtypes.bfloat16)
    # [p, c*32 + j]: j<16 -> whi[c*128+p, j], j>=16 -> wlo[c*128+p, j-16]
    gw2 = np.concatenate(
        [whi.reshape(E, 128, E), wlo.reshape(E, 128, E)], axis=2
    )  # [chunk, p, 32]
    gw2 = np.ascontiguousarray(gw2.transpose(1, 0, 2).reshape(128, E * 32))

    # fcw[de, e*512+h] = fc_w[e, h, de]
    fcw = np.ascontiguousarray(
        fc_w.transpose(2, 0, 1).reshape(128, E * H)
    ).astype(ml_dtypes.bfloat16)
    # pjw[p, (e*4+hc)*128+d] = proj_w[e, d, hc*128+p]
    pjw = np.ascontiguousarray(
        proj_w.reshape(E, DE, 4, 128).transpose(3, 0, 2, 1).reshape(128, E * 4 * DE)
    ).astype(ml_dtypes.bfloat16)
    ngb = np.ascontiguousarray(np.broadcast_to(np.tile(-gate_b, NTAU), (128, NTAU * E)))
    idn = np.ascontiguousarray(np.vstack([np.eye(16), np.eye(16)]).astype(np.float32))
    fcb = np.ascontiguousarray(
        fc_b.reshape(E, 4, 128).transpose(2, 0, 1).reshape(128, E * 4)
    )
    pjb = np.ascontiguousarray(
        np.broadcast_to(proj_b.reshape(1, E * DE), (128, E * DE))
    ).astype(ml_dtypes.bfloat16)

    shared = {
        "gw2": gw2,
        "fcw": fcw,
        "pjw": pjw,
        "ngb": ngb,
        "idn": idn,
        "fcb": fcb,
        "pjb": pjb,
    }
    in_maps = []
    for i in range(NCORES):
        xc = x[i * TPC : (i + 1) * TPC]  # [TPC, C]
        # xT[c'=e*128+d, t] = x[t, 16*d+e]
        xT = np.ascontiguousarray(
            xc.reshape(TPC, DE, E).transpose(2, 1, 0).reshape(C, TPC)
        )
        xhi = xT.astype(ml_dtypes.bfloat16)
        xlo = (xT - xhi.astype(np.float32)).astype(ml_dtypes.bfloat16)
        in_maps.append({"xhi": xhi, "xlo": xlo, **shared})
    return in_maps


def kernel(x, gate_w, gate_b, fc_w, fc_b, proj_w, proj_b, _trace=False, _tmpdir=None):
    fc_b = np.asarray(fc_b, dtype=np.float32)
    proj_b = np.asarray(proj_b, dtype=np.float32)
    has_fcb = bool(np.any(fc_b != 0))
    has_pjb = bool(np.any(proj_b != 0))
    key = ("nc", has_fcb, has_pjb)
    if key not in _CACHE:
        _CACHE[key] = _build(has_fcb, has_pjb)
    nc = _CACHE[key]
    in_maps = _prep_inputs(x, gate_w, gate_b, fc_w, fc_b, proj_w, proj_b)
    res = run_bass_kernel_spmd(
        nc,
        in_maps,
        core_ids=list(range(NCORES)),
        trace=_trace,
        tmpdir=_tmpdir,
    )
    out = np.concatenate(
        [np.asarray(res.results[i]["out"]) for i in range(NCORES)], axis=0
    )
    out = out.astype(np.float32).reshape(B, T, C)
    if _trace:
        _CACHE["last_result"] = res
    return out
